# revision 19
# baseline (speedup 1.0000x reference)
"""Multi-head attention (B=2, S=2048, D=1024, H=16) on 8 Trainium2 NeuronCores.

Sharding: 2-way data parallel over batch x 4-way tensor parallel over heads.
Core c handles batch c//4 and heads [4*(c%4), 4*(c%4)+4).  Each core computes
its 4 heads' attention and a partial output projection; the host sums the 4
partials per batch element (the bias bo is only added by the g==0 cores).

Schedule: flash-style jq-outer rounds with a fill scheduler.  The attention
blocks (score matmul -> exp -> att@V accumulate) form the steady beat; every
projection matmul (K/V/Q quarters, output projection) is a "fill unit"
interleaved between blocks so the PE never idles waiting on the Act engine's
exp, and the Act engine never idles behind a burst of projection matmuls.
x inputs travel as bf16 (halves input DMA); weights and the attention path
stay float32r.
"""

from contextlib import ExitStack

import numpy as np
import ml_dtypes

import concourse.mybir as mybir
import concourse.tile as tile
from concourse import bacc
from concourse import bass_utils
from concourse._compat import with_exitstack

F32 = mybir.dt.float32
F32R = mybir.dt.float32r
BF16 = mybir.dt.bfloat16

SB_DT = BF16          # attention path in bf16 (fast PE weight loads)
X_DT = BF16           # x activations on the wire + in SBUF
W_DT = BF16           # projection weights on the wire + in SBUF
W_NP = ml_dtypes.bfloat16
X_NP = ml_dtypes.bfloat16

D_MODEL = 1024
N_HEAD = 16
DK = 64
B = 2
S = 2048
N_CORES = 8
HPC = 4          # heads per core
DPC = HPC * DK   # 256 output dims per core
KC = D_MODEL // 128   # 8 contraction chunks of 128
SQ = 512         # sequence quarter
NSQ = S // SQ    # 4
NJB = S // 128   # 16 key blocks


@with_exitstack
def build_mha(ctx: ExitStack, tc, ins, out_ap, loop_n=None):
    """Emit the per-core kernel.  loop_n wraps the whole compute body in a
    hardware For_i loop (used only for timing measurement)."""
    nc = tc.nc
    P = 128
    Exp = mybir.ActivationFunctionType.Exp
    Add = mybir.AluOpType.add

    xq = ins["xq_t"].rearrange("(kc p) s -> p kc s", p=P)
    xk = ins["xk_t"].rearrange("(kc p) s -> p kc s", p=P)
    xv = ins["xv_t"].rearrange("(kc p) s -> p kc s", p=P)
    out = out_ap.rearrange("(sb p) n -> p sb n", p=P)

    ec = ctx.enter_context
    cpool = ec(tc.tile_pool(name="consts", bufs=1))
    xpool = ec(tc.tile_pool(name="xs", bufs=5))
    qkpool = ec(tc.tile_pool(name="qk", bufs=1))
    vpool = ec(tc.tile_pool(name="vh", bufs=1))
    ptpool = ec(tc.tile_pool(name="pt", bufs=6))
    apool = ec(tc.tile_pool(name="attn", bufs=1))
    opool = ec(tc.tile_pool(name="outs", bufs=2))
    npool = ec(tc.tile_pool(name="nrm", bufs=10))
    accpool = ec(tc.tile_pool(name="acc", bufs=1))
    pp_ps = ec(tc.tile_pool(name="proj_ps", bufs=1, space="PSUM"))
    sc_ps = ec(tc.tile_pool(name="score_ps", bufs=2, space="PSUM"))
    at_ps = ec(tc.tile_pool(name="att_ps", bufs=2, space="PSUM"))

    # --- constants ---
    wq_sb = cpool.tile([P, KC, DPC], W_DT, tag="wq")
    wk_sb = cpool.tile([P, KC, DPC], W_DT, tag="wk")
    wv_sb = cpool.tile([P, KC, DPC], W_DT, tag="wv")
    wo_sb = cpool.tile([P, 2, D_MODEL], W_DT, tag="wo")
    # per-chunk weight loads on the scalar-engine DMA queue: the first K/V
    # projection matmuls only wait for their own chunk; x tiles stream on the
    # sync queue in parallel.
    wq_ap = ins["wq_t"].rearrange("(kc p) m -> p kc m", p=P)
    wk_ap = ins["wk_t"].rearrange("(kc p) m -> p kc m", p=P)
    wv_ap = ins["wv_t"].rearrange("(kc p) m -> p kc m", p=P)
    nc.scalar.dma_start(wk_sb[:], wk_ap[:])
    nc.scalar.dma_start(wv_sb[:], wv_ap[:])
    nc.scalar.dma_start(wq_sb[:], wq_ap[:])
    bq_sb = cpool.tile([P, 2], F32, tag="bq")
    bk_sb = cpool.tile([P, 2], F32, tag="bk")
    bv_sb = cpool.tile([P, DPC], F32, tag="bv")
    bo_sb = cpool.tile([P, D_MODEL], F32, tag="bo")
    nc.gpsimd.dma_start(bq_sb[:], ins["bq_p"][:])
    nc.gpsimd.dma_start(bk_sb[:], ins["bk_p"][:])
    nc.gpsimd.dma_start(bv_sb[:], ins["bv_b"][:])

    # --- persistent activations ---
    qh_sb = qkpool.tile([P, 2, S], SB_DT, tag="qh")   # [dk%128, head_pair, s]
    kh_sb = qkpool.tile([P, 2, S], SB_DT, tag="kh")
    vh_sb = vpool.tile([P, NJB, HPC, DK + 1], SB_DT, tag="vh")  # + ones col
    at_sb = apool.tile([P, 2, S], W_DT, tag="at")    # attn out, transposed

    # walrus can't memset float32r; memset f32 then broadcast-copy
    ones1 = cpool.tile([P, 1], F32, tag="ones1")
    nc.vector.memset(ones1[:], 1.0)
    ones_r = cpool.tile([1, 64], F32R, tag="ones_r")
    nc.vector.tensor_copy(ones_r[:], ones1[0:1, :].to_broadcast((1, 64)))
    nc.vector.tensor_copy(
        vh_sb[:, :, :, DK : DK + 1],
        ones1[:, None, None, :].to_broadcast((P, NJB, HPC, 1)),
    )

    # attention partial accumulators, one per (head, query-quarter);
    # row 64 carries the running sum(exp) for the softmax denominator
    acc_sb = [
        [accpool.tile([65, 512], F32, tag=f"acc{i5}_{h}", name=f"acc{i5}_{h}")
         for h in range(HPC)]
        for i5 in range(NSQ)
    ]

    def _compute():
        # ---- fill scheduler ----
        # fill holds (deadline, earliest, emit_fn) in deadline order; a unit
        # is one short burst of PE work (1-8 matmuls) plus its DMA/DVE
        # bookkeeping.  blk[0] is the global attention-block counter.
        fill = []
        blk = [0]

        def drain(dl):
            while fill and fill[0][0] <= dl:
                fill.pop(0)[2]()

        def pop_one():
            for i, (dl, earliest, fn) in enumerate(fill):
                if earliest <= blk[0]:
                    fill.pop(i)
                    fn()
                    return

        def qk_units(x_ap, w_sb, b_sb, dst, sq, dma):
            st = {}

            def mk(kc):
                def run():
                    if kc == 0:
                        st["ps"] = pp_ps.tile([P, 1024], F32, tag="pp", name="pp")
                        xt = xpool.tile([P, KC, SQ], X_DT, tag="xt", name="xt")
                        dma.dma_start(
                            xt[:], x_ap[:, :, sq * SQ : (sq + 1) * SQ])
                        st["xt"] = xt
                    ps = st["ps"]
                    xt = st["xt"]
                    nc.tensor.matmul(
                        ps[:, 0:512], w_sb[:, kc, 0:128], xt[:, kc, :],
                        start=(kc == 0), stop=(kc == KC - 1),
                    )
                    nc.tensor.matmul(
                        ps[:, 512:1024], w_sb[:, kc, 128:256], xt[:, kc, :],
                        start=(kc == 0), stop=(kc == KC - 1),
                    )
                    if kc == KC - 1:
                        i_sl = slice(sq * SQ, (sq + 1) * SQ)
                        nc.vector.tensor_scalar_add(
                            dst[:, 0, i_sl], ps[:, 0:512], b_sb[:, 0:1]
                        )
                        nc.vector.tensor_scalar_add(
                            dst[:, 1, i_sl], ps[:, 512:1024], b_sb[:, 1:2]
                        )

                return run

            return [mk(kc) for kc in range(KC)]

        def v_units(sq, dma):
            st = {}

            def dma_unit():
                xt = xpool.tile([P, KC, SQ], X_DT, tag="xt", name="xt")
                dma.dma_start(xt[:], xv[:, :, sq * SQ : (sq + 1) * SQ])
                st["xt"] = xt

            def mk(sbi):
                def run():
                    if sbi == 0:
                        st["ps"] = pp_ps.tile([P, 1024], F32, tag="pp", name="pp")
                    ps = st["ps"]
                    for kc in range(KC):
                        nc.tensor.matmul(
                            ps[:, sbi * 256 : (sbi + 1) * 256],
                            st["xt"][:, kc, sbi * 128 : (sbi + 1) * 128],
                            wv_sb[:, kc, :],
                            start=(kc == 0), stop=(kc == KC - 1),
                        )
                    jb = sq * 4 + sbi
                    nc.vector.tensor_tensor(
                        vh_sb[:, jb, :, 0:DK],
                        ps[:, sbi * 256 : (sbi + 1) * 256].rearrange(
                            "p (h d) -> p h d", h=HPC
                        ),
                        bv_sb[:].rearrange("p (h d) -> p h d", h=HPC),
                        Add,
                    )

                return run

            return [dma_unit] + [mk(sbi) for sbi in range(4)]

        def final_units(i5, ps_pool=None, ps_tag="pp"):
            st = {}
            pool = ps_pool or pp_ps

            def mk(sbi):
                def run():
                    if sbi % 2 == 0:
                        st["ot"] = opool.tile(
                            [P, 2, 1024], F32, tag="ot", name="ot")
                    sb = i5 * 4 + sbi
                    s_sl = slice(sb * 128, (sb + 1) * 128)
                    po = pool.tile([P, 1024], F32, tag=ps_tag, name="po")
                    for c in range(2):
                        nc.tensor.matmul(
                            po[:, 0:512], at_sb[:, c, s_sl], wo_sb[:, c, 0:512],
                            start=(c == 0), stop=(c == 1),
                        )
                        nc.tensor.matmul(
                            po[:, 512:1024], at_sb[:, c, s_sl],
                            wo_sb[:, c, 512:1024],
                            start=(c == 0), stop=(c == 1),
                        )
                    ot = st["ot"]
                    nc.vector.tensor_tensor(ot[:, sbi % 2, :], po[:], bo_sb[:], Add)
                    if sbi % 2 == 1:
                        nc.sync.dma_start(
                            out[:, sb - 1 : sb + 1, :], ot[:])

                return run

            return [mk(sbi) for sbi in range(4)]

        def _normalize(i5):
            # all 4 heads' chains emitted stage-parallel: recips back-to-back
            # on DVE, partition-broadcasts as engine-free DMAs, then muls
            i_sl = slice(i5 * SQ, (i5 + 1) * SQ)
            rcs, bcs = [], []
            for h in range(HPC):
                rc = npool.tile([1, 512], F32R, tag="rc", name="rc")
                with nc.allow_low_precision(reason="f32r recip for PE bcast"):
                    nc.vector.reciprocal(rc[:], acc_sb[i5][h][64:65, :])
                rcs.append(rc)
            for h in range(HPC):
                # broadcast row across 64 partitions via a tiny PE matmul
                # into an att-pool PSUM tile (free right after the acc drain)
                bct = at_ps.tile([P, 512], F32, tag="att", name="bc")
                nc.tensor.matmul(
                    bct[0:64, :], ones_r[:], rcs[h][:], start=True, stop=True)
                bcs.append(bct)
            for h in range(HPC):
                acc, bc, t = acc_sb[i5][h], bcs[h][0:64, :], h // 2
                if h % 2 == 0:
                    nc.vector.tensor_mul(at_sb[0:64, t, i_sl], acc[0:64, :], bc)
                else:
                    tm = npool.tile([64, 512], W_DT, tag="tm", name="tm")
                    nc.vector.tensor_mul(tm[:], acc[0:64, :], bc)
                    nc.gpsimd.dma_start(at_sb[64:128, t, i_sl], tm[:])

        def attn_group(i5, t, jq):
            """4 key-blocks of attention for head pair t, query quarter i5,
            with fill units interleaved between blocks."""
            i_sl = slice(i5 * SQ, (i5 + 1) * SQ)
            att_e = at_ps.tile([P, 512], F32, tag="att")
            att_o = at_ps.tile([P, 512], F32, tag="att")
            pts = []
            jbs = range(jq * 4, jq * 4 + 4)
            for n, jb in enumerate(jbs):
                drain((jq, (blk[0] % 32)))
                sc = sc_ps.tile([P, 1024], F32, tag="sc")
                j_sl = slice(jb * 128, (jb + 1) * 128)
                nc.tensor.matmul(
                    sc[:, 0:512], kh_sb[0:64, t, j_sl], qh_sb[0:64, t, i_sl],
                    start=True, stop=True,
                )
                nc.tensor.matmul(
                    sc[:, 512:1024], kh_sb[64:128, t, j_sl],
                    qh_sb[64:128, t, i_sl], start=True, stop=True,
                )
                pt = ptpool.tile([P, 1024], SB_DT, tag="pt")
                nc.scalar.activation(pt[:], sc[:], Exp, scale=1.0 / np.sqrt(DK))
                pts.append(pt)
                pop_one()
                blk[0] += 1
                if n > 0:
                    ptp = pts[n - 1]
                    nc.tensor.matmul(
                        att_e[0:65, :], vh_sb[:, jb - 1, 2 * t, :],
                        ptp[:, 0:512], start=(n - 1 == 0), stop=False,
                    )
                    nc.tensor.matmul(
                        att_o[0:65, :], vh_sb[:, jb - 1, 2 * t + 1, :],
                        ptp[:, 512:1024], start=(n - 1 == 0), stop=False,
                    )
            jb_last = jq * 4 + 3
            nc.tensor.matmul(
                att_e[0:65, :], vh_sb[:, jb_last, 2 * t, :],
                pts[-1][:, 0:512], start=False, stop=True,
            )
            nc.tensor.matmul(
                att_o[0:65, :], vh_sb[:, jb_last, 2 * t + 1, :],
                pts[-1][:, 512:1024], start=False, stop=True,
            )
            for h, aps in ((2 * t, att_e), (2 * t + 1, att_o)):
                acc = acc_sb[i5][h]
                if jq == 0:
                    nc.vector.tensor_copy(acc[:], aps[0:65, :])
                else:
                    nc.vector.tensor_tensor(acc[:], acc[:], aps[0:65, :], Add)

        # ---- prologue: K0 + Q0 on the sync queue, V0 on gpsimd (behind
        # only the small biases); wo/bo queued last (needed only at round 3)
        for u in qk_units(xk, wk_sb, bk_sb, kh_sb, 0, nc.sync):
            u()
        for n, u in enumerate(v_units(0, nc.gpsimd)):
            if n == 0:
                u()  # xv0 quarter DMA issues immediately
            else:
                fill.append(((0, n - 1), 0, u))
        for u in qk_units(xq, wq_sb, bq_sb, qh_sb, 0, nc.sync):
            u()
        nc.gpsimd.dma_start(
            wo_sb[:], ins["wo_t"].rearrange("(c p) n -> p c n", p=P))
        nc.gpsimd.dma_start(bo_sb[:], ins["bo_b"][:])

        # ---- fill queue in deadline order ----
        # deadlines (round, block-in-round) force-drain at the latest safe
        # point; earliest spreads pops so late rounds are not starved of
        # fill and DMA queues are not flooded early.
        for sq in range(1, NSQ):
            for u in qk_units(xq, wq_sb, bq_sb, qh_sb, sq, nc.sync):
                fill.append(((0, sq * 8 - 2), (sq - 1) * 6, u))
        for sq in range(1, NSQ):
            for u in qk_units(xk, wk_sb, bk_sb, kh_sb, sq, nc.scalar):
                fill.append(((sq, 0), (sq - 1) * 32 + 14, u))
            for u in v_units(sq, nc.scalar):
                fill.append(((sq, 0), (sq - 1) * 32 + 18, u))
        fill.sort(key=lambda x: x[0])

        # ---- rounds ----
        for jq in range(NSQ):
            drain((jq, 0))
            for i5 in range(NSQ):
                for t in range(2):
                    attn_group(i5, t, jq)
                if jq == NSQ - 1:
                    _normalize(i5)
                    if i5 < NSQ - 1:
                        # pace finals 2 blocks apart: each unit occupies the
                        # single pp PSUM buffer ~2.3us (4 matmuls + DVE drain)
                        for k, u in enumerate(final_units(i5)):
                            fill.append(((99, 0), blk[0] + 2 * k, u))
                    else:
                        # last quarter: nothing left to hide behind; use the
                        # now-idle score PSUM pool for a 2-deep po rotation
                        for u in final_units(
                            i5, ps_pool=sc_ps, ps_tag="sc"
                        ):
                            u()
        drain((100, 0))

    if loop_n is not None and loop_n > 1:
        with tc.For_i(0, loop_n, 1):
            _compute()
    else:
        _compute()


def shard_inputs(q, k, v, Wq, bq, Wk, bk, Wv, bv, Wo, bo):
    """Build the 8 per-core input maps from the full inputs."""

    def prep_w(a):
        return np.ascontiguousarray(np.asarray(a, np.float32)).astype(W_NP)

    def prep_x(a):
        return np.ascontiguousarray(np.asarray(a, np.float32)).astype(X_NP)

    in_maps = []
    for c in range(N_CORES):
        b, g = divmod(c, 4)
        hs = slice(g * DPC, (g + 1) * DPC)
        bo_b = (
            np.broadcast_to(np.asarray(bo, np.float32), (128, D_MODEL))
            if g == 0
            else np.zeros((128, D_MODEL), np.float32)
        )
        in_maps.append({
            "xq_t": prep_x(np.asarray(q)[b].T),
            "xk_t": prep_x(np.asarray(k)[b].T),
            "xv_t": prep_x(np.asarray(v)[b].T),
            "wq_t": prep_w(np.asarray(Wq)[hs, :].T),
            "wk_t": prep_w(np.asarray(Wk)[hs, :].T),
            "wv_t": prep_w(np.asarray(Wv)[hs, :].T),
            "wo_t": prep_w(np.asarray(Wo)[:, hs].T),
            "bq_p": np.ascontiguousarray(
                np.asarray(bq, np.float32)[hs].reshape(2, 128).T),
            "bk_p": np.ascontiguousarray(
                np.asarray(bk, np.float32)[hs].reshape(2, 128).T),
            "bv_b": np.ascontiguousarray(
                np.broadcast_to(np.asarray(bv, np.float32)[hs], (128, DPC))),
            "bo_b": np.ascontiguousarray(bo_b),
        })
    return in_maps


_NC = None


def build_nc(loop_n=None):
    nc = bacc.Bacc(
        "TRN2",
        target_bir_lowering=False,
        debug=False,
        enable_asserts=False,
        num_devices=N_CORES,
    )
    ins = {}
    for name in ("xq_t", "xk_t", "xv_t"):
        ins[name] = nc.dram_tensor(
            name, [D_MODEL, S], X_DT, kind="ExternalInput").ap()
    for name in ("wq_t", "wk_t", "wv_t"):
        ins[name] = nc.dram_tensor(
            name, [D_MODEL, DPC], W_DT, kind="ExternalInput").ap()
    ins["wo_t"] = nc.dram_tensor(
        "wo_t", [DPC, D_MODEL], W_DT, kind="ExternalInput").ap()
    ins["bq_p"] = nc.dram_tensor("bq_p", [128, 2], F32, kind="ExternalInput").ap()
    ins["bk_p"] = nc.dram_tensor("bk_p", [128, 2], F32, kind="ExternalInput").ap()
    ins["bv_b"] = nc.dram_tensor("bv_b", [128, DPC], F32, kind="ExternalInput").ap()
    ins["bo_b"] = nc.dram_tensor(
        "bo_b", [128, D_MODEL], F32, kind="ExternalInput").ap()
    out_ap = nc.dram_tensor("out", [S, D_MODEL], F32, kind="ExternalOutput").ap()
    with tile.TileContext(nc) as tc:
        build_mha(tc, ins, out_ap, loop_n=loop_n)
    nc.compile()
    return nc


def _get_nc():
    global _NC
    if _NC is None:
        _NC = build_nc()
    return _NC


def run_sharded(inputs, trace=False):
    nc = _get_nc()
    in_maps = shard_inputs(**inputs)
    res = bass_utils.run_bass_kernel_spmd(
        nc, in_maps, core_ids=list(range(N_CORES)), trace=trace
    )
    acc = np.zeros((B, S, D_MODEL), np.float64)
    for c in range(N_CORES):
        acc[c // 4] += res.results[c]["out"].astype(np.float64)
    return acc.astype(np.float32), res


def kernel(**inputs):
    out, _ = run_sharded(inputs, trace=False)
    return out
